# revision 5
# baseline (speedup 1.0000x reference)
"""NT-Xent contrastive loss on 8 Trainium2 NeuronCores (bf16 + XBAR).

Math (reference): z = [z_i; z_j] (N=8192, D=128), zn = z/||z||,
sim = zn@zn.T / 0.1.  Row loss_i = logsumexp_{j!=i} sim[i,j] - sim[i, pos(i)],
loss = mean_i loss_i.

Sharding: rolled-column trick.  Core c receives z rolled by -1024*c rows.
Its 1024 local rows are rolled rows 0..1023; in rolled coordinates the
self column of local row i is i and the positive column is i + 4096 on
EVERY core, so a single static SPMD program works with no collectives.
The self logit is suppressed by adding -5 to the diagonal cosine
(logit -40 -> exp ~4e-18, negligible).  Host sums the 8 partial means.

V2 design (from the measured ~105us V1):
  - The ACT exp stream is the floor (1 elem/cycle/lane @1.2GHz, dtype
    independent).  V2 offloads DVE_W of each 2048-col chunk to DVE via a
    16-bit Schraudolph exp: i16 = int(A*cos + B) bitcast to bf16 IS
    exp(10*cos) to ~2% (tolerance is 2e-2 on a heavily averaged scalar);
    a second DVE pass reduces the bf16 view into per-chunk row sums.
  - Prologue restructured for pipelining: batch-0 arrives as four
    separate 512-row tensors (DMA dep tracking is whole-tensor) on four
    queues; ssq/scale run per-512-row piece; znT is built as per-half
    tensors, halves xbar'd on sync + scalar in parallel; the first
    matmul starts after half 0.
  - Stage-A (square/reduce/norm/scale/xbar) for batch b+1 runs under
    column chunk b, spread across r=1..6, deprioritized so the list
    scheduler cannot hoist it into the critical prologue.
Known-dead ends (trace-verified in V1): collectives (~87us latency),
full-f32 Schraudolph (3 passes), finer prologue splits without priority
pinning (scheduler inversion).
"""

import os
import sys

import numpy as np

_TRN_REPO = "/opt/trn_rl_repo"
if _TRN_REPO not in sys.path:
    sys.path.insert(0, _TRN_REPO)

from concourse import bacc, bass, mybir, tile
from concourse.bass_utils import run_bass_kernel_spmd

B = 4096
D = 128
N = 2 * B
N_CORES = 8
RPC = N // N_CORES  # 1024 rows per core
INV_T = 10.0
DIAG_SHIFT = -5.0

NBATCH = 4  # stage-A batches of 2048 rows
TPB = 16    # 128-row tiles per batch
RB = 8      # row blocks per core (128 rows each)
QB = 4      # 2048-wide column chunks
KB = 4      # 512-wide matmuls per chunk

DVE_W = 576             # columns of each chunk exp'd on DVE (Schraudolph)
ACT_W = 2048 - DVE_W    # columns exp'd on ACT
# Schraudolph constants: bf16 bit pattern of exp(10*x) ~= int(A*x + B)
SCH_A = INV_T * 128.0 / float(np.log(2.0))
SCH_B = 127.0 * 128.0 - 7.4

_cache: dict = {}


def build():
    f32 = mybir.dt.float32
    bf16 = mybir.dt.bfloat16
    i16 = mybir.dt.int16
    AX = mybir.AxisListType
    AF = mybir.ActivationFunctionType
    ALU = mybir.AluOpType

    nc = bacc.Bacc(
        "TRN2", target_bir_lowering=False, debug=False, num_devices=N_CORES
    )

    # Pin ln/exp/copy/etc to one ACT table: avoids 1.3us ACT_TABLE_LOAD at
    # every ln<->exp transition.
    tabs = bacc.get_activation_tables(nc.m.arch)
    pinned = set(tabs["natural_log_exp_and_others"])
    for k in tabs:
        if k != "natural_log_exp_and_others":
            tabs[k] = tabs[k] - pinned

    # z arrives as bf16 (host casts during the roll/shard prep): halves the
    # HBM read traffic all 8 cores contend for.
    z_dram = nc.dram_tensor("z_roll", [N, D], bf16, kind="ExternalInput")
    loss_dram = nc.dram_tensor("loss_part", [1, 1], f32, kind="ExternalOutput")

    eye_np = np.eye(128, dtype=np.float32)
    eye_dram = nc.inline_tensor(eye_np, name="eye128")
    negI_dram = nc.inline_tensor(
        (DIAG_SHIFT * eye_np).astype(np.float32), name="negI128"
    )
    ones_dram = nc.inline_tensor(np.ones((128, 1), np.float32), name="ones128")

    with tile.TileContext(nc) as tc:
        with (
            tc.tile_pool(name="const", bufs=1) as cpool,
            tc.tile_pool(name="zin", bufs=NBATCH) as zpool,
            tc.tile_pool(name="zn", bufs=2) as npool,
            tc.tile_pool(name="persist", bufs=1) as ppool,
            tc.tile_pool(name="scr", bufs=2) as spool,
            tc.tile_pool(name="psum", bufs=2, space=bass.MemorySpace.PSUM) as qpool,
        ):
            eye_sb = cpool.tile([128, 128], f32)
            negI_sb = cpool.tile([128, 128], f32)
            ones_sb = cpool.tile([128, 1], f32)

            # bf16 ssq: 0.4% error on ||z||^2 -> 0.2% on the norm, noise on
            # the final averaged scalar; buys DVE 2x on the reduce.
            ssq = ppool.tile([128, NBATCH * TPB], bf16)
            lnssq = ppool.tile([128, NBATCH * TPB], f32)
            inv = ppool.tile([128, NBATCH * TPB], f32)
            # znT as per-half tensors: DMA writes are whole-tensor granular,
            # so per-half tensors let matmuls start after half 0's xbar.
            znT = [
                [
                    ppool.tile([128, 1024], bf16, name=f"znT{b}h{h}")
                    for h in range(2)
                ]
                for b in range(NBATCH)
            ]
            sexp = ppool.tile([128, RB, QB], f32)
            sexp_d = ppool.tile([128, RB, QB], f32)
            pos = ppool.tile([128, RB], f32)

            # --- input DMAs ---
            # batch 0 as four separate 512-row tensors on four queues: the
            # per-piece stage-A chain starts as each piece lands instead of
            # waiting for the whole batch.
            zin0 = [
                zpool.tile([128, 4, 128], bf16, name=f"zin0s{s}")
                for s in range(4)
            ]
            b0_engs = [nc.sync, nc.scalar, nc.gpsimd, nc.sync]
            for s in range(4):
                r0 = 512 * s
                src = z_dram[r0 : r0 + 512, :].rearrange(
                    "(t p) d -> p t d", p=128
                )
                b0_engs[s].dma_start(zin0[s][:], src)

            zin_tiles = {}
            for b in range(1, NBATCH):
                zin_tiles[b] = zpool.tile(
                    [128, TPB, 128], bf16, name=f"zin{b}"
                )

            # batches 1-3 + constants gated behind batch 0's arrival via
            # dummy gpsimd reads so they don't steal DMA bandwidth from the
            # critical prologue chain.
            gate = cpool.tile([128, 16], bf16)
            for s in range(4):
                nc.gpsimd.tensor_copy(
                    gate[:, 4 * s : 4 * s + 4], zin0[s][:, 3, 0:4]
                )
            for b in range(1, NBATCH):
                for s in range(4):
                    r0 = 2048 * b + 512 * s
                    src = z_dram[r0 : r0 + 512, :].rearrange(
                        "(t p) d -> p t d", p=128
                    )
                    nc.gpsimd.dma_start(
                        zin_tiles[b][:, 4 * s : 4 * s + 4, :], src
                    )
            nc.gpsimd.dma_start(eye_sb[:], eye_dram[:])
            nc.gpsimd.dma_start(negI_sb[:], negI_dram[:])
            nc.gpsimd.dma_start(ones_sb[:], ones_dram[:])

            def bc(iv):
                # broadcast [128, t] -> [128, t, 128] via stride-0 last dim
                return bass.AP(iv.tensor, iv.offset, iv.ap + [[0, 128]])

            # --- prologue: batch 0 per-piece stage-A at top priority ---
            zn_tiles = {}
            zn_tiles[0] = npool.tile([128, TPB, 128], bf16, name="zn0", tag="zn")
            with tc.high_priority():
                for s in range(4):
                    scr_s = spool.tile([128, 4 * 128], bf16, tag="sqp")
                    zv = zin0[s][:].rearrange("p t d -> p (t d)")
                    nc.vector.tensor_mul(scr_s[:], zv, zv)
                    with nc.allow_low_precision(reason="bf16 ssq, 0.4% ok"):
                        nc.vector.reduce_sum(
                            ssq[:, 4 * s : 4 * s + 4],
                            scr_s[:].rearrange("p (t d) -> p t d", d=128),
                            axis=AX.X,
                        )
                    if s % 2 == 1:
                        # norms for this half: 1/||z|| = exp(-0.5*ln(ssq))
                        h = s // 2
                        j0, j1 = 8 * h, 8 * h + 8
                        nc.scalar.activation(
                            lnssq[:, j0:j1], ssq[:, j0:j1], AF.Ln
                        )
                        nc.scalar.activation(
                            inv[:, j0:j1], lnssq[:, j0:j1], AF.Exp, scale=-0.5
                        )
                for s in range(4):
                    nc.vector.tensor_mul(
                        zn_tiles[0][:, 4 * s : 4 * s + 4, :],
                        zin0[s][:],
                        bc(inv[:, 4 * s : 4 * s + 4]),
                    )
                    if s % 2 == 1:
                        h = s // 2
                        eng = nc.sync if h == 0 else nc.scalar
                        eng.dma_start_transpose(
                            znT[0][h][:].rearrange("p (t c) -> p t c", c=128),
                            zn_tiles[0][:, 8 * h : 8 * h + 8, :].rearrange(
                                "p t d -> p (t d)"
                            ),
                        )

            # --- in-loop stage-A helpers for batches 1-3 ---
            def ssq_mul(b):
                scr = spool.tile([128, TPB * 128], bf16, tag="sq")
                zv = zin_tiles[b][:].rearrange("p t d -> p (t d)")
                nc.vector.tensor_mul(scr[:], zv, zv)
                return scr

            def ssq_red(b, scr):
                j0 = TPB * b
                with nc.allow_low_precision(reason="bf16 ssq, 0.4% ok"):
                    nc.vector.reduce_sum(
                        ssq[:, j0 : j0 + TPB],
                        scr[:].rearrange("p (t d) -> p t d", d=128),
                        axis=AX.X,
                    )

            def norms(b):
                j0 = TPB * b
                nc.scalar.activation(
                    lnssq[:, j0 : j0 + TPB], ssq[:, j0 : j0 + TPB], AF.Ln
                )
                nc.scalar.activation(
                    inv[:, j0 : j0 + TPB], lnssq[:, j0 : j0 + TPB],
                    AF.Exp, scale=-0.5,
                )

            def tsm(b, h):
                if b not in zn_tiles:
                    zn_tiles[b] = npool.tile(
                        [128, TPB, 128], bf16, name=f"zn{b}", tag="zn"
                    )
                t0, t1 = 8 * h, 8 * h + 8
                nc.vector.tensor_mul(
                    zn_tiles[b][:, t0:t1, :],
                    zin_tiles[b][:, t0:t1, :],
                    bc(inv[:, TPB * b + t0 : TPB * b + t1]),
                )

            def build_trans(b, h):
                nc.sync.dma_start_transpose(
                    znT[b][h][:].rearrange("p (t c) -> p t c", c=128),
                    zn_tiles[b][:, 8 * h : 8 * h + 8, :].rearrange(
                        "p t d -> p (t d)"
                    ),
                )

            # --- main loop: q-outer, r-inner; build batch q+1 under chunk q ---
            for q in range(QB):
                b = q + 1
                for r in range(RB):
                    lhsT = znT[0][0][:, 128 * r : 128 * (r + 1)]
                    ps = qpool.tile([128, 2048], f32, tag="mm")
                    for k in range(KB):
                        rhs = znT[q][k // 2][:, 512 * (k % 2) : 512 * (k % 2 + 1)]
                        nc.tensor.matmul(
                            ps[:, 512 * k : 512 * (k + 1)],
                            lhsT,
                            rhs,
                            start=True,
                            stop=True,
                        )
                    if q == 0:
                        sub = ps[:, 128 * r : 128 * (r + 1)]
                        nc.vector.tensor_add(sub, sub, negI_sb[:])
                    if q == 2:
                        scr = spool.tile([128, 128], f32, tag="pos")
                        nc.vector.tensor_mul(
                            scr[:], ps[:, 128 * r : 128 * (r + 1)], eye_sb[:]
                        )
                        nc.vector.reduce_sum(
                            pos[:, r : r + 1], scr[:], axis=AX.X
                        )
                    # ACT: exp with fused row-sum accumulate on ACT_W cols
                    nc.scalar.activation(
                        ps[:, 0:ACT_W],
                        ps[:, 0:ACT_W],
                        AF.Exp,
                        scale=INV_T,
                        accum_out=sexp[:, r, q : q + 1],
                    )
                    # DVE: Schraudolph exp on the last DVE_W cols + reduce
                    e16 = spool.tile([128, DVE_W], i16, tag="sch")
                    nc.vector.tensor_scalar(
                        e16[:],
                        ps[:, ACT_W:2048],
                        SCH_A,
                        SCH_B,
                        op0=ALU.mult,
                        op1=ALU.add,
                    )
                    nc.vector.reduce_sum(
                        sexp_d[:, r, q : q + 1],
                        e16[:].bitcast(bf16),
                        axis=AX.X,
                    )
                    if b < NBATCH:
                        # deprioritized: the list scheduler otherwise hoists
                        # this ready DVE work between the prologue's
                        # producers and consumers, delaying the first exp.
                        with tc.high_priority(offset=-100000):
                            if r == 1:
                                zn_tiles[f"scr{b}"] = ssq_mul(b)
                            elif r == 2:
                                ssq_red(b, zn_tiles[f"scr{b}"])
                            elif r == 3:
                                norms(b)
                            elif r == 4:
                                tsm(b, 0)
                                tsm(b, 1)
                            elif r == 5:
                                build_trans(b, 0)
                            elif r == 6:
                                build_trans(b, 1)

            # --- epilogue ---
            s8 = ppool.tile([128, RB], f32)
            s8d = ppool.tile([128, RB], f32)
            nc.vector.reduce_sum(s8[:], sexp[:], axis=AX.X)
            nc.vector.reduce_sum(s8d[:], sexp_d[:], axis=AX.X)
            nc.vector.tensor_add(s8[:], s8[:], s8d[:])
            possum = ppool.tile([128, 1], f32)
            nc.vector.reduce_sum(possum[:], pos[:], axis=AX.X)
            lse = ppool.tile([128, RB], f32)
            lnsum = ppool.tile([128, 1], f32)
            nc.scalar.activation(lse[:], s8[:], AF.Ln, accum_out=lnsum[:])
            tot = ppool.tile([128, 1], f32)
            nc.vector.tensor_scalar(
                tot[:],
                possum[:],
                -INV_T,
                None,
                op0=ALU.mult,
            )
            nc.vector.tensor_add(tot[:], tot[:], lnsum[:])
            psf = qpool.tile([128, 2048], f32, tag="mm")
            nc.tensor.matmul(
                psf[0:1, 0:1], ones_sb[:], tot[:], start=True, stop=True
            )
            res = ppool.tile([1, 1], f32)
            nc.scalar.mul(res[:], psf[0:1, 0:1], 1.0 / N)
            nc.gpsimd.dma_start(loss_dram[:], res[:])

    nc.compile()
    return nc


def get_nc():
    if "nc" not in _cache:
        _cache["nc"] = build()
    return _cache["nc"]


def make_in_maps(z_i: np.ndarray, z_j: np.ndarray):
    import ml_dtypes

    z = np.concatenate(
        [np.asarray(z_i, np.float32), np.asarray(z_j, np.float32)], axis=0
    ).astype(ml_dtypes.bfloat16)
    return [
        {"z_roll": np.ascontiguousarray(np.roll(z, -RPC * c, axis=0))}
        for c in range(N_CORES)
    ]


def kernel(**inputs) -> np.ndarray:
    in_maps = make_in_maps(inputs["z_i"], inputs["z_j"])
    nc = get_nc()
    res = run_bass_kernel_spmd(nc, in_maps, list(range(N_CORES)))
    kernel.last_results = res
    total = np.float32(0.0)
    for r in res.results:
        total = np.float32(total + np.float32(np.asarray(r["loss_part"]).reshape(())))
    return np.float32(total)
